# revision 1
# baseline (speedup 1.0000x reference)
"""Causal self-attention (B=2, T=2048, C=2048, H=16, Dh=128) on 8 TRN2 NeuronCores.

Sharding: dp=2 over batch x tp=4 over heads (4 heads/core).
  - c_attn column-parallel: each core holds W_attn columns for its 4 heads
    (q, k, v slices), computes qT/kT (head-dim major) and v directly from
    a host-pre-transposed, pre-tiled xT.
  - attention: per-head causal blocks, scoresT in (k, q) orientation; the
    softmax denominator comes free from a ones column appended to v, and
    the exp'd attT blocks feed matmul2 as stationary weights (no
    transposes in the attention inner loop).
  - c_proj row-parallel: each core computes its partial y_heads @ W_proj
    rows; the 4 partials per batch are summed on the host (unshard).

The three stages are software-pipelined per 512-row T-chunk, ordered
q,k-projections -> head-0 scores+exp -> v-projections -> remaining
heads -> deferred c_proj of the previous chunk, so ScalarE exp work
hides under TensorE matmul work of the neighboring sub-stages.

All matmuls run in bf16 (inputs pre-cast on host), accumulation fp32.
Host pre-tiles every input so all DMAs are fully contiguous.
"""

import numpy as np
import ml_dtypes

import concourse.bass as bass
import concourse.tile as tile
from concourse import bacc, mybir
from concourse.bass_utils import run_bass_kernel_spmd
from concourse.masks import make_identity, make_upper_triangular

BF16 = mybir.dt.bfloat16
F32 = mybir.dt.float32

B, T, C = 2, 2048, 2048
N_HEAD, D_HEAD = 16, 128
P = 128
KT = C // P          # 16 contraction tiles for qkv projection
NH = 4               # heads per core (tp=4)
TCH = 4              # T chunks of 512 (pipeline granularity)
VW = 129             # v width with appended ones column
WG = 4               # kt-groups per input DMA (finer grain to start PE early)
SCALE = float(1.0 / np.sqrt(D_HEAD))

_CACHE: dict = {}


def _build_program(repeat: int | None = None) -> bacc.Bacc:
    """Build the SPMD program. With `repeat`, the whole body runs inside a
    hardware For_i loop (used only for timing measurements)."""
    import contextlib
    nc = bacc.Bacc("TRN2", target_bir_lowering=False, debug=False)

    # host-pre-tiled layouts (all DMAs contiguous):
    #   xT:  (TCH, P, KT, 512)  xT[tc, p, kt, t'] = x[b][tc*512+t', kt*128+p]
    #   wq/wk/wv: (P, KT, 512)  w[p, kt, m] = W[kt*128+p, m]
    #   wp:  (P, NH, C)         wp[p, h, n] = W_proj[g*512 + h*128+p, n]
    xT_h = nc.dram_tensor("xT", (TCH, P, KT, 512), BF16, kind="ExternalInput")
    wq_h = nc.dram_tensor("wq", (P, KT, NH * P), BF16, kind="ExternalInput")
    wk_h = nc.dram_tensor("wk", (P, KT, NH * P), BF16, kind="ExternalInput")
    wv_h = nc.dram_tensor("wv", (P, KT, NH * P), BF16, kind="ExternalInput")
    wp_h = nc.dram_tensor("wp", (P, NH, C), BF16, kind="ExternalInput")
    out_h = nc.dram_tensor("out", (T, C), F32, kind="ExternalOutput")

    xT_d = xT_h.ap()
    wq_d, wk_d, wv_d, wp_d = wq_h.ap(), wk_h.ap(), wv_h.ap(), wp_h.ap()
    out_d = out_h.ap().rearrange("(mt p) n -> p mt n", p=P)    # (128, 16, 2048)

    with tile.TileContext(nc) as tc_:
        with (
            tc_.tile_pool(name="consts", bufs=1) as consts,
            tc_.tile_pool(name="persist", bufs=1) as persist,
            tc_.tile_pool(name="wpool", bufs=1) as wpool,
            tc_.tile_pool(name="xpool", bufs=2) as xpool,
            tc_.tile_pool(name="attp", bufs=21) as attp,
            tc_.tile_pool(name="ytp", bufs=2) as ytp,
            tc_.tile_pool(name="ynorm", bufs=3) as ynorm,
            tc_.tile_pool(name="osb", bufs=2) as osb,
            tc_.tile_pool(name="psA", bufs=2, space="PSUM") as psA,
            tc_.tile_pool(name="psS", bufs=2, space="PSUM") as psS,
            tc_.tile_pool(name="psY", bufs=1, space="PSUM") as psY,
            tc_.tile_pool(name="psT", bufs=1, space="PSUM") as psT,
            tc_.tile_pool(name="psO", bufs=2, space="PSUM") as psO,
        ):
            ident = consts.tile([P, P], BF16)
            tri = consts.tile([P, P], BF16)
            make_identity(nc, ident)
            # tri[i, j] = 1 where i <= j (keep k <= q), else 0
            make_upper_triangular(nc, tri, val=1.0, diag=True)

            qT = [persist.tile([P, T], BF16, tag=f"qT{h}", name=f"qT{h}")
                  for h in range(NH)]
            kTt = [persist.tile([P, T], BF16, tag=f"kT{h}", name=f"kT{h}")
                   for h in range(NH)]
            # v_aug for all heads: va_all[:, kk, h, 0:128]=v, [...,128]=1.0
            va_all = persist.tile([P, KT, NH, VW], BF16, tag="va")
            wp_sb = persist.tile([P, NH, C], BF16, tag="wp")

            wq_sb = wpool.tile([P, KT, NH * P], BF16, tag="wq")
            wk_sb = wpool.tile([P, KT, NH * P], BF16, tag="wk")
            wv_sb = wpool.tile([P, KT, NH * P], BF16, tag="wv")

            # first x chunk + q/k weights first (kt-group granularity) so the
            # first matmuls start as early as possible
            xc0 = xpool.tile([P, KT, 512], BF16, tag="xc", name="xc0")
            KG = KT // WG
            for g in range(WG):
                gs = slice(g * KG, (g + 1) * KG)
                nc.sync.dma_start(xc0[:, gs], xT_d[0, :, gs])
                nc.sync.dma_start(wq_sb[:, gs], wq_d[:, gs])
            for g in range(WG):
                gs = slice(g * KG, (g + 1) * KG)
                nc.sync.dma_start(wk_sb[:, gs], wk_d[:, gs])
            for g in range(WG):
                gs = slice(g * KG, (g + 1) * KG)
                nc.sync.dma_start(wv_sb[:, gs], wv_d[:, gs])
            nc.sync.dma_start(wp_sb[:], wp_d[:])
            # ones column for free softmax denominators
            nc.vector.memset(va_all[:, :, :, D_HEAD:VW], 1.0)

            loop_ctx = tc_.For_i(0, repeat, 1) if repeat else contextlib.nullcontext()

            def emit_proj(tcu, ytc):
                # ---- Stage C(tc): c_proj partial for rows of chunk tc ----
                # emitted after B(tc+1) so it backfills PE during exp stalls
                for j in range(4):
                    mt = tcu * 4 + j
                    o = osb.tile([P, C], F32, tag="o", name=f"o_{mt}")
                    for nck in range(4):
                        ns = slice(nck * 512, (nck + 1) * 512)
                        po = psO.tile([P, 512], F32, tag="psO", name=f"psO_{mt}_{nck}")
                        for h in range(NH):
                            nc.tensor.matmul(
                                po, ytc[h][:, j * P:(j + 1) * P], wp_sb[:, h, ns],
                                start=(h == 0), stop=(h == NH - 1),
                            )
                        nc.vector.tensor_copy(o[:, ns], po)
                    nc.sync.dma_start(out_d[:, mt, :], o[:])

            with loop_ctx:
                prev_ytc = None
                xcs = xc0
                for tcu in range(TCH):
                    ts = slice(tcu * 512, (tcu + 1) * 512)
                    xc = xcs
                    if tcu + 1 < TCH:  # prefetch next chunk
                        xcs = xpool.tile([P, KT, 512], BF16, tag="xc",
                                         name=f"xc{tcu + 1}")
                        nc.sync.dma_start(xcs[:], xT_d[tcu + 1])

                    # ---- Stage A(tc): q,k projections first ----
                    for h in range(NH):
                        hs = slice(h * P, (h + 1) * P)
                        pq = psA.tile([P, 512], F32, tag="psA")
                        for kk in range(KT):
                            nc.tensor.matmul(
                                pq, wq_sb[:, kk, hs], xc[:, kk, :],
                                start=(kk == 0), stop=(kk == KT - 1),
                            )
                        nc.vector.tensor_copy(qT[h][:, ts], pq)
                    for h in range(NH):
                        hs = slice(h * P, (h + 1) * P)
                        pk = psA.tile([P, 512], F32, tag="psA")
                        for kk in range(KT):
                            nc.tensor.matmul(
                                pk, wk_sb[:, kk, hs], xc[:, kk, :],
                                start=(kk == 0), stop=(kk == KT - 1),
                            )
                        nc.vector.tensor_copy(kTt[h][:, ts], pk)

                    qc = tcu

                    def attn_scores(h):
                        # mm1 + exp (+ causal mask) for one head's q chunk
                        att = []
                        for kk in range(4 * qc + 4):
                            ps = psS.tile([P, 512], F32, tag="psS")
                            a = attp.tile([P, 512], BF16, tag="att")
                            if kk < 4 * qc:
                                nc.tensor.matmul(
                                    ps, kTt[h][:, kk * P:(kk + 1) * P], qT[h][:, ts],
                                    start=True, stop=True,
                                )
                                nc.scalar.activation(
                                    a[:], ps[:],
                                    mybir.ActivationFunctionType.Exp, scale=SCALE,
                                )
                            else:
                                off = (kk - 4 * qc) * P
                                nc.tensor.matmul(
                                    ps[:, off:512],
                                    kTt[h][:, kk * P:(kk + 1) * P],
                                    qT[h][:, qc * 512 + off:(qc + 1) * 512],
                                    start=True, stop=True,
                                )
                                nc.scalar.activation(
                                    a[:, off:512], ps[:, off:512],
                                    mybir.ActivationFunctionType.Exp, scale=SCALE,
                                )
                                nc.vector.tensor_tensor(
                                    a[:, off:off + P], a[:, off:off + P], tri,
                                    mybir.AluOpType.mult,
                                )
                            att.append(a)
                        return att

                    # head 0 scores/exp pulled forward: ScalarE works on them
                    # while PE does the v-projections below
                    att0 = attn_scores(0)

                    # ---- Stage A(tc): v projections ----
                    for tt in range(4):
                        pv = psA.tile([P, 512], F32, tag="psA")
                        for kk in range(KT):
                            nc.tensor.matmul(
                                pv, xc[:, kk, tt * P:(tt + 1) * P], wv_sb[:, kk, :],
                                start=(kk == 0), stop=(kk == KT - 1),
                            )
                        nc.vector.tensor_copy(
                            va_all[:, tcu * 4 + tt, :, 0:D_HEAD],
                            pv.rearrange("p (h d) -> p h d", h=NH),
                        )

                    # ---- Stage B(qc=tc): remaining heads + weighted sums ----
                    ytc = []
                    for h in range(NH):
                        att = att0 if h == 0 else attn_scores(h)

                        yt = ytp.tile([P, 512], BF16, tag=f"yT{h}", name=f"yT{h}_{qc}")
                        ytc.append(yt)
                        for j in range(4):
                            qt = qc * 4 + j
                            py = psY.tile([P, VW], F32, tag="psY")
                            for kk in range(qt + 1):
                                nc.tensor.matmul(
                                    py, att[kk][:, j * P:(j + 1) * P],
                                    va_all[:, kk, h, :],
                                    start=(kk == 0), stop=(kk == qt),
                                )
                            r = ynorm.tile([P, 1], F32, tag="r")
                            nc.vector.reciprocal(r, py[:, D_HEAD:VW])
                            y = ynorm.tile([P, P], BF16, tag="y")
                            nc.vector.tensor_scalar_mul(y, py[:, 0:D_HEAD], r)
                            pt = psT.tile([P, P], BF16, tag="psT")
                            nc.tensor.transpose(pt, y, ident)
                            nc.vector.tensor_copy(yt[:, j * P:(j + 1) * P], pt)

                    if prev_ytc is not None:
                        emit_proj(tcu - 1, prev_ytc)
                    prev_ytc = ytc

                emit_proj(TCH - 1, prev_ytc)

    nc.compile()
    return nc


def _get_program() -> bacc.Bacc:
    if "nc" not in _CACHE:
        _CACHE["nc"] = _build_program()
    return _CACHE["nc"]


def _make_in_maps(x, W_attn, W_proj):
    bf = ml_dtypes.bfloat16
    x = np.asarray(x, dtype=np.float32)
    W_attn = np.asarray(W_attn, dtype=np.float32)
    W_proj = np.asarray(W_proj, dtype=np.float32)

    # xT[tc, p, kt, t'] = x[b][tc*512+t', kt*128+p]
    xT_b = []
    for b in range(B):
        xt = x[b].T.reshape(KT, P, TCH, 512).transpose(2, 1, 0, 3)
        xT_b.append(np.ascontiguousarray(xt).astype(bf))

    def _tile_w(w):  # (C, 512) -> (P, KT, 512)
        return np.ascontiguousarray(
            w.reshape(KT, P, NH * P).transpose(1, 0, 2)).astype(bf)

    GW = NH * D_HEAD  # 512 columns per tp group
    in_maps = []
    for core in range(8):
        b, g = divmod(core, 4)
        wp = W_proj[g * GW:(g + 1) * GW, :].reshape(NH, P, C).transpose(1, 0, 2)
        in_maps.append({
            "xT": xT_b[b],
            "wq": _tile_w(W_attn[:, g * GW:(g + 1) * GW]),
            "wk": _tile_w(W_attn[:, C + g * GW: C + (g + 1) * GW]),
            "wv": _tile_w(W_attn[:, 2 * C + g * GW: 2 * C + (g + 1) * GW]),
            "wp": np.ascontiguousarray(wp).astype(bf),
        })
    return in_maps


def kernel(x, W_attn, W_proj, _want_results=False, _trace=False):
    nc = _get_program()
    in_maps = _make_in_maps(x, W_attn, W_proj)
    res = run_bass_kernel_spmd(
        nc, in_maps, core_ids=list(range(8)), trace=_trace,
    )
    parts = [np.asarray(res.results[i]["out"], dtype=np.float32) for i in range(8)]
    out = np.stack([
        parts[0] + parts[1] + parts[2] + parts[3],
        parts[4] + parts[5] + parts[6] + parts[7],
    ]).astype(np.float32)
    if _want_results:
        return out, res
    return out



# revision 2
# speedup vs baseline: 1.0765x; 1.0765x over previous
"""Causal self-attention (B=2, T=2048, C=2048, H=16, Dh=128) on 8 TRN2 NeuronCores.

Sharding: dp=2 over batch x tp=4 over heads (4 heads/core).
  - c_attn column-parallel: each core holds W_attn columns for its 4 heads
    (q, k, v slices), computes qT/kT (head-dim major) and v directly from
    a host-pre-transposed, pre-tiled xT.
  - attention: per-head causal blocks, scoresT in (k, q) orientation; the
    softmax denominator comes free from a ones column appended to v, and
    the exp'd attT blocks feed matmul2 as stationary weights (no
    transposes in the attention inner loop).
  - c_proj row-parallel: each core computes its partial y_heads @ W_proj
    rows; the 4 partials per batch are summed on the host (unshard).

The three stages are software-pipelined per 512-row T-chunk, ordered
q,k-projections -> head-0 scores+exp -> v-projections -> remaining
heads -> deferred c_proj of the previous chunk, so ScalarE exp work
hides under TensorE matmul work of the neighboring sub-stages.

All matmuls run in bf16 (inputs pre-cast on host), accumulation fp32.
Host pre-tiles every input so all DMAs are fully contiguous.
"""

import numpy as np
import ml_dtypes

import concourse.bass as bass
import concourse.tile as tile
from concourse import bacc, mybir
from concourse.bass_utils import run_bass_kernel_spmd
from concourse.masks import make_identity, make_upper_triangular

BF16 = mybir.dt.bfloat16
F32 = mybir.dt.float32

B, T, C = 2, 2048, 2048
N_HEAD, D_HEAD = 16, 128
P = 128
KT = C // P          # 16 contraction tiles for qkv projection
NH = 4               # heads per core (tp=4)
TCH = 4              # T chunks of 512 (pipeline granularity)
VW = 129             # v width with appended ones column
WG = 4               # kt-groups per input DMA (finer grain to start PE early)
SCALE = float(1.0 / np.sqrt(D_HEAD))

_CACHE: dict = {}


def _build_program(repeat: int | None = None) -> bacc.Bacc:
    """Build the SPMD program. With `repeat`, the whole body runs inside a
    hardware For_i loop (used only for timing measurements)."""
    import contextlib
    nc = bacc.Bacc("TRN2", target_bir_lowering=False, debug=False)

    # host-pre-tiled layouts (all DMAs contiguous):
    #   xT:  (TCH, P, KT, 512)  xT[tc, p, kt, t'] = x[b][tc*512+t', kt*128+p]
    #   wq/wk/wv: (P, KT, 512)  w[p, kt, m] = W[kt*128+p, m]
    #   wp:  (P, NH, C)         wp[p, h, n] = W_proj[g*512 + h*128+p, n]
    xT_h = nc.dram_tensor("xT", (TCH, P, KT, 512), BF16, kind="ExternalInput")
    wq_h = nc.dram_tensor("wq", (P, KT, NH * P), BF16, kind="ExternalInput")
    wk_h = nc.dram_tensor("wk", (P, KT, NH * P), BF16, kind="ExternalInput")
    wv_h = nc.dram_tensor("wv", (P, KT, NH * P), BF16, kind="ExternalInput")
    wp_h = nc.dram_tensor("wp", (P, NH, C), BF16, kind="ExternalInput")
    out_h = nc.dram_tensor("out", (T, C), BF16, kind="ExternalOutput")

    xT_d = xT_h.ap()
    wq_d, wk_d, wv_d, wp_d = wq_h.ap(), wk_h.ap(), wv_h.ap(), wp_h.ap()
    out_d = out_h.ap().rearrange("(mt p) n -> p mt n", p=P)    # (128, 16, 2048)

    with tile.TileContext(nc) as tc_:
        with (
            tc_.tile_pool(name="consts", bufs=1) as consts,
            tc_.tile_pool(name="persist", bufs=1) as persist,
            tc_.tile_pool(name="wpool", bufs=1) as wpool,
            tc_.tile_pool(name="xpool", bufs=2) as xpool,
            tc_.tile_pool(name="attp", bufs=21) as attp,
            tc_.tile_pool(name="ytp", bufs=2) as ytp,
            tc_.tile_pool(name="ynorm", bufs=3) as ynorm,
            tc_.tile_pool(name="osb", bufs=2) as osb,
            tc_.tile_pool(name="psA", bufs=2, space="PSUM") as psA,
            tc_.tile_pool(name="psS", bufs=2, space="PSUM") as psS,
            tc_.tile_pool(name="psY", bufs=1, space="PSUM") as psY,
            tc_.tile_pool(name="psT", bufs=1, space="PSUM") as psT,
            tc_.tile_pool(name="psO", bufs=2, space="PSUM") as psO,
        ):
            ident = consts.tile([P, P], BF16)
            tri = consts.tile([P, P], BF16)
            make_identity(nc, ident)
            # tri[i, j] = 1 where i <= j (keep k <= q), else 0
            make_upper_triangular(nc, tri, val=1.0, diag=True)

            qT = [persist.tile([P, T], BF16, tag=f"qT{h}", name=f"qT{h}")
                  for h in range(NH)]
            kTt = [persist.tile([P, T], BF16, tag=f"kT{h}", name=f"kT{h}")
                   for h in range(NH)]
            # v_aug for all heads: va_all[:, kk, h, 0:128]=v, [...,128]=1.0
            va_all = persist.tile([P, KT, NH, VW], BF16, tag="va")
            wp_sb = persist.tile([P, NH, C], BF16, tag="wp")

            wq_sb = wpool.tile([P, KT, NH * P], BF16, tag="wq")
            wk_sb = wpool.tile([P, KT, NH * P], BF16, tag="wk")
            wv_sb = wpool.tile([P, KT, NH * P], BF16, tag="wv")

            # first x chunk + q/k weights first (kt-group granularity) so the
            # first matmuls start as early as possible
            xc0 = xpool.tile([P, KT, 512], BF16, tag="xc", name="xc0")
            KG = KT // WG
            for g in range(WG):
                gs = slice(g * KG, (g + 1) * KG)
                nc.sync.dma_start(xc0[:, gs], xT_d[0, :, gs])
                nc.sync.dma_start(wq_sb[:, gs], wq_d[:, gs])
            for g in range(WG):
                gs = slice(g * KG, (g + 1) * KG)
                nc.sync.dma_start(wk_sb[:, gs], wk_d[:, gs])
            for g in range(WG):
                gs = slice(g * KG, (g + 1) * KG)
                nc.sync.dma_start(wv_sb[:, gs], wv_d[:, gs])
            nc.sync.dma_start(wp_sb[:], wp_d[:])
            # ones column for free softmax denominators
            nc.vector.memset(va_all[:, :, :, D_HEAD:VW], 1.0)

            loop_ctx = tc_.For_i(0, repeat, 1) if repeat else contextlib.nullcontext()

            def emit_proj(tcu, ytc):
                # ---- Stage C(tc): c_proj partial for rows of chunk tc ----
                # emitted after B(tc+1) so it backfills PE during exp stalls
                for j in range(4):
                    mt = tcu * 4 + j
                    o = osb.tile([P, C], BF16, tag="o", name=f"o_{mt}")
                    for nck in range(4):
                        ns = slice(nck * 512, (nck + 1) * 512)
                        po = psO.tile([P, 512], F32, tag="psO", name=f"psO_{mt}_{nck}")
                        for h in range(NH):
                            nc.tensor.matmul(
                                po, ytc[h][:, j * P:(j + 1) * P], wp_sb[:, h, ns],
                                start=(h == 0), stop=(h == NH - 1),
                            )
                        nc.vector.tensor_copy(o[:, ns], po)
                    nc.sync.dma_start(out_d[:, mt, :], o[:])

            with loop_ctx:
                prev_ytc = None
                xcs = xc0
                for tcu in range(TCH):
                    ts = slice(tcu * 512, (tcu + 1) * 512)
                    xc = xcs
                    if tcu + 1 < TCH:  # prefetch next chunk
                        xcs = xpool.tile([P, KT, 512], BF16, tag="xc",
                                         name=f"xc{tcu + 1}")
                        nc.sync.dma_start(xcs[:], xT_d[tcu + 1])

                    # ---- Stage A(tc): q,k projections first ----
                    for h in range(NH):
                        hs = slice(h * P, (h + 1) * P)
                        pq = psA.tile([P, 512], F32, tag="psA")
                        for kk in range(KT):
                            nc.tensor.matmul(
                                pq, wq_sb[:, kk, hs], xc[:, kk, :],
                                start=(kk == 0), stop=(kk == KT - 1),
                            )
                        nc.vector.tensor_copy(qT[h][:, ts], pq)
                    for h in range(NH):
                        hs = slice(h * P, (h + 1) * P)
                        pk = psA.tile([P, 512], F32, tag="psA")
                        for kk in range(KT):
                            nc.tensor.matmul(
                                pk, wk_sb[:, kk, hs], xc[:, kk, :],
                                start=(kk == 0), stop=(kk == KT - 1),
                            )
                        nc.vector.tensor_copy(kTt[h][:, ts], pk)

                    qc = tcu

                    def attn_scores(h):
                        # mm1 + exp (+ causal mask) for one head's q chunk
                        att = []
                        for kk in range(4 * qc + 4):
                            ps = psS.tile([P, 512], F32, tag="psS")
                            a = attp.tile([P, 512], BF16, tag="att")
                            if kk < 4 * qc:
                                nc.tensor.matmul(
                                    ps, kTt[h][:, kk * P:(kk + 1) * P], qT[h][:, ts],
                                    start=True, stop=True,
                                )
                                nc.scalar.activation(
                                    a[:], ps[:],
                                    mybir.ActivationFunctionType.Exp, scale=SCALE,
                                )
                            else:
                                off = (kk - 4 * qc) * P
                                nc.tensor.matmul(
                                    ps[:, off:512],
                                    kTt[h][:, kk * P:(kk + 1) * P],
                                    qT[h][:, qc * 512 + off:(qc + 1) * 512],
                                    start=True, stop=True,
                                )
                                nc.scalar.activation(
                                    a[:, off:512], ps[:, off:512],
                                    mybir.ActivationFunctionType.Exp, scale=SCALE,
                                )
                                nc.vector.tensor_tensor(
                                    a[:, off:off + P], a[:, off:off + P], tri,
                                    mybir.AluOpType.mult,
                                )
                            att.append(a)
                        return att

                    # head 0 scores/exp pulled forward: ScalarE works on them
                    # while PE does the v-projections below
                    att0 = attn_scores(0)

                    # ---- Stage A(tc): v projections ----
                    for tt in range(4):
                        pv = psA.tile([P, 512], F32, tag="psA")
                        for kk in range(KT):
                            nc.tensor.matmul(
                                pv, xc[:, kk, tt * P:(tt + 1) * P], wv_sb[:, kk, :],
                                start=(kk == 0), stop=(kk == KT - 1),
                            )
                        nc.vector.tensor_copy(
                            va_all[:, tcu * 4 + tt, :, 0:D_HEAD],
                            pv.rearrange("p (h d) -> p h d", h=NH),
                        )

                    # ---- Stage B(qc=tc): remaining heads + weighted sums ----
                    ytc = []
                    for h in range(NH):
                        att = att0 if h == 0 else attn_scores(h)

                        yt = ytp.tile([P, 512], BF16, tag=f"yT{h}", name=f"yT{h}_{qc}")
                        ytc.append(yt)
                        for j in range(4):
                            qt = qc * 4 + j
                            py = psY.tile([P, VW], F32, tag="psY")
                            for kk in range(qt + 1):
                                nc.tensor.matmul(
                                    py, att[kk][:, j * P:(j + 1) * P],
                                    va_all[:, kk, h, :],
                                    start=(kk == 0), stop=(kk == qt),
                                )
                            r = ynorm.tile([P, 1], F32, tag="r")
                            nc.vector.reciprocal(r, py[:, D_HEAD:VW])
                            y = ynorm.tile([P, P], BF16, tag="y")
                            nc.vector.tensor_scalar_mul(y, py[:, 0:D_HEAD], r)
                            pt = psT.tile([P, P], BF16, tag="psT")
                            nc.tensor.transpose(pt, y, ident)
                            nc.vector.tensor_copy(yt[:, j * P:(j + 1) * P], pt)

                    if prev_ytc is not None:
                        emit_proj(tcu - 1, prev_ytc)
                    prev_ytc = ytc

                emit_proj(TCH - 1, prev_ytc)

    nc.compile()
    return nc


def _get_program() -> bacc.Bacc:
    if "nc" not in _CACHE:
        _CACHE["nc"] = _build_program()
    return _CACHE["nc"]


def _make_in_maps(x, W_attn, W_proj):
    bf = ml_dtypes.bfloat16
    x = np.asarray(x, dtype=np.float32)
    W_attn = np.asarray(W_attn, dtype=np.float32)
    W_proj = np.asarray(W_proj, dtype=np.float32)

    # xT[tc, p, kt, t'] = x[b][tc*512+t', kt*128+p]
    xT_b = []
    for b in range(B):
        xt = x[b].T.reshape(KT, P, TCH, 512).transpose(2, 1, 0, 3)
        xT_b.append(np.ascontiguousarray(xt).astype(bf))

    def _tile_w(w):  # (C, 512) -> (P, KT, 512)
        return np.ascontiguousarray(
            w.reshape(KT, P, NH * P).transpose(1, 0, 2)).astype(bf)

    GW = NH * D_HEAD  # 512 columns per tp group
    in_maps = []
    for core in range(8):
        b, g = divmod(core, 4)
        wp = W_proj[g * GW:(g + 1) * GW, :].reshape(NH, P, C).transpose(1, 0, 2)
        in_maps.append({
            "xT": xT_b[b],
            "wq": _tile_w(W_attn[:, g * GW:(g + 1) * GW]),
            "wk": _tile_w(W_attn[:, C + g * GW: C + (g + 1) * GW]),
            "wv": _tile_w(W_attn[:, 2 * C + g * GW: 2 * C + (g + 1) * GW]),
            "wp": np.ascontiguousarray(wp).astype(bf),
        })
    return in_maps


def kernel(x, W_attn, W_proj, _want_results=False, _trace=False):
    nc = _get_program()
    in_maps = _make_in_maps(x, W_attn, W_proj)
    res = run_bass_kernel_spmd(
        nc, in_maps, core_ids=list(range(8)), trace=_trace,
    )
    parts = [np.asarray(res.results[i]["out"], dtype=np.float32) for i in range(8)]
    out = np.stack([
        parts[0] + parts[1] + parts[2] + parts[3],
        parts[4] + parts[5] + parts[6] + parts[7],
    ]).astype(np.float32)
    if _want_results:
        return out, res
    return out



# revision 3
# speedup vs baseline: 1.2772x; 1.1864x over previous
"""Causal self-attention (B=2, T=2048, C=2048, H=16, Dh=128) on 8 TRN2 NeuronCores.

Sharding: dp=2 over batch x tp=4 over heads (4 heads/core).
  - c_attn column-parallel: each core holds W_attn columns for its 4 heads
    (q, k, v slices), computes qT/kT (head-dim major) and v directly from
    a host-pre-transposed, pre-tiled xT.
  - attention: per-head causal blocks, scoresT in (k, q) orientation; the
    softmax denominator comes free from a ones column appended to v, and
    the exp'd attT blocks feed matmul2 as stationary weights (no
    transposes in the attention inner loop).
  - c_proj row-parallel: each core computes its partial y_heads @ W_proj
    rows; the 4 partials per batch are summed on the host (unshard).

The three stages are software-pipelined per 512-row T-chunk, ordered
q,k-projections -> head-0 scores+exp -> v-projections -> remaining
heads -> deferred c_proj of the previous chunk, so ScalarE exp work
hides under TensorE matmul work of the neighboring sub-stages.

All matmuls run in bf16 (inputs pre-cast on host), accumulation fp32.
Host pre-tiles every input so all DMAs are fully contiguous.
"""

import numpy as np
import ml_dtypes

import concourse.bass as bass
import concourse.tile as tile
from concourse import bacc, mybir
from concourse.bass_utils import run_bass_kernel_spmd
from concourse.masks import make_identity, make_upper_triangular

BF16 = mybir.dt.bfloat16
F32 = mybir.dt.float32

B, T, C = 2, 2048, 2048
N_HEAD, D_HEAD = 16, 128
P = 128
KT = C // P          # 16 contraction tiles for qkv projection
NH = 4               # heads per core (tp=4)
TCH = 4              # T chunks of 512 (pipeline granularity)
VW = 129             # v width with appended ones column
WG = 4               # kt-groups per input DMA (finer grain to start PE early)
SCALE = float(1.0 / np.sqrt(D_HEAD))

_CACHE: dict = {}


def _build_program(repeat: int | None = None) -> bacc.Bacc:
    """Build the SPMD program. With `repeat`, the whole body runs inside a
    hardware For_i loop (used only for timing measurements)."""
    import contextlib
    nc = bacc.Bacc("TRN2", target_bir_lowering=False, debug=False)

    # host-pre-tiled layouts (all DMAs contiguous):
    #   xT:  (TCH, P, KT, 512)  xT[tc, p, kt, t'] = x[b][tc*512+t', kt*128+p]
    #   wq/wk/wv: (P, KT, 512)  w[p, kt, m] = W[kt*128+p, m]
    #   wp:  (P, NH, C)         wp[p, h, n] = W_proj[g*512 + h*128+p, n]
    xT_h = nc.dram_tensor("xT", (TCH, P, KT, 512), BF16, kind="ExternalInput")
    wq_h = nc.dram_tensor("wq", (P, KT, NH * P), BF16, kind="ExternalInput")
    wk_h = nc.dram_tensor("wk", (P, KT, NH * P), BF16, kind="ExternalInput")
    wv_h = nc.dram_tensor("wv", (P, KT, NH * P), BF16, kind="ExternalInput")
    wp_h = nc.dram_tensor("wp", (P, NH, C), BF16, kind="ExternalInput")
    out_h = nc.dram_tensor("out", (T, C), F32, kind="ExternalOutput")

    xT_d = xT_h.ap()
    wq_d, wk_d, wv_d, wp_d = wq_h.ap(), wk_h.ap(), wv_h.ap(), wp_h.ap()
    out_d = out_h.ap().rearrange("(mt p) n -> p mt n", p=P)    # (128, 16, 2048)

    with tile.TileContext(nc) as tc_:
        with (
            tc_.tile_pool(name="consts", bufs=1) as consts,
            tc_.tile_pool(name="persist", bufs=1) as persist,
            tc_.tile_pool(name="wpool", bufs=1) as wpool,
            tc_.tile_pool(name="xpool", bufs=2) as xpool,
            tc_.tile_pool(name="attp", bufs=21) as attp,
            tc_.tile_pool(name="ytp", bufs=2) as ytp,
            tc_.tile_pool(name="ynorm", bufs=3) as ynorm,
            tc_.tile_pool(name="osb", bufs=2) as osb,
            tc_.tile_pool(name="psA", bufs=2, space="PSUM") as psA,
            tc_.tile_pool(name="psS", bufs=2, space="PSUM") as psS,
            tc_.tile_pool(name="psY", bufs=1, space="PSUM") as psY,
            tc_.tile_pool(name="psT", bufs=1, space="PSUM") as psT,
            tc_.tile_pool(name="psO", bufs=2, space="PSUM") as psO,
        ):
            ident = consts.tile([P, P], BF16)
            tri = consts.tile([P, P], BF16)
            make_identity(nc, ident)
            # tri[i, j] = 1 where i <= j (keep k <= q), else 0
            make_upper_triangular(nc, tri, val=1.0, diag=True)

            qT = [persist.tile([P, T], BF16, tag=f"qT{h}", name=f"qT{h}")
                  for h in range(NH)]
            kTt = [persist.tile([P, T], BF16, tag=f"kT{h}", name=f"kT{h}")
                   for h in range(NH)]
            # v_aug for all heads: va_all[:, kk, h, 0:128]=v, [...,128]=1.0
            va_all = persist.tile([P, KT, NH, VW], BF16, tag="va")
            wp_sb = persist.tile([P, NH, C], BF16, tag="wp")

            wq_sb = wpool.tile([P, KT, NH * P], BF16, tag="wq")
            wk_sb = wpool.tile([P, KT, NH * P], BF16, tag="wk")
            wv_sb = wpool.tile([P, KT, NH * P], BF16, tag="wv")

            # first x chunk + q/k weights first (kt-group granularity) so the
            # first matmuls start as early as possible
            xc0 = xpool.tile([P, KT, 512], BF16, tag="xc", name="xc0")
            KG = KT // WG
            for g in range(WG):
                gs = slice(g * KG, (g + 1) * KG)
                nc.sync.dma_start(xc0[:, gs], xT_d[0, :, gs])
                nc.sync.dma_start(wq_sb[:, gs], wq_d[:, gs])
            for g in range(WG):
                gs = slice(g * KG, (g + 1) * KG)
                nc.sync.dma_start(wk_sb[:, gs], wk_d[:, gs])
            for g in range(WG):
                gs = slice(g * KG, (g + 1) * KG)
                nc.sync.dma_start(wv_sb[:, gs], wv_d[:, gs])
            nc.sync.dma_start(wp_sb[:], wp_d[:])
            # ones column for free softmax denominators
            nc.vector.memset(va_all[:, :, :, D_HEAD:VW], 1.0)

            loop_ctx = tc_.For_i(0, repeat, 1) if repeat else contextlib.nullcontext()

            def emit_proj(tcu, ytc):
                # ---- Stage C(tc): c_proj partial for rows of chunk tc ----
                # emitted after B(tc+1) so it backfills PE during exp stalls
                for j in range(4):
                    mt = tcu * 4 + j
                    o = osb.tile([P, C], F32, tag="o", name=f"o_{mt}")
                    for nck in range(4):
                        ns = slice(nck * 512, (nck + 1) * 512)
                        po = psO.tile([P, 512], F32, tag="psO", name=f"psO_{mt}_{nck}")
                        for h in range(NH):
                            nc.tensor.matmul(
                                po, ytc[h][:, j * P:(j + 1) * P], wp_sb[:, h, ns],
                                start=(h == 0), stop=(h == NH - 1),
                            )
                        nc.vector.tensor_copy(o[:, ns], po)
                    nc.sync.dma_start(out_d[:, mt, :], o[:])

            with loop_ctx:
                prev_ytc = None
                xcs = xc0
                for tcu in range(TCH):
                    ts = slice(tcu * 512, (tcu + 1) * 512)
                    xc = xcs
                    if tcu + 1 < TCH:  # prefetch next chunk
                        xcs = xpool.tile([P, KT, 512], BF16, tag="xc",
                                         name=f"xc{tcu + 1}")
                        nc.sync.dma_start(xcs[:], xT_d[tcu + 1])

                    # ---- Stage A(tc): q,k projections first ----
                    for h in range(NH):
                        hs = slice(h * P, (h + 1) * P)
                        pq = psA.tile([P, 512], F32, tag="psA")
                        for kk in range(KT):
                            nc.tensor.matmul(
                                pq, wq_sb[:, kk, hs], xc[:, kk, :],
                                start=(kk == 0), stop=(kk == KT - 1),
                            )
                        nc.vector.tensor_copy(qT[h][:, ts], pq)
                    for h in range(NH):
                        hs = slice(h * P, (h + 1) * P)
                        pk = psA.tile([P, 512], F32, tag="psA")
                        for kk in range(KT):
                            nc.tensor.matmul(
                                pk, wk_sb[:, kk, hs], xc[:, kk, :],
                                start=(kk == 0), stop=(kk == KT - 1),
                            )
                        nc.vector.tensor_copy(kTt[h][:, ts], pk)

                    qc = tcu

                    def attn_scores(h):
                        # mm1 + exp (+ causal mask) for one head's q chunk
                        att = []
                        for kk in range(4 * qc + 4):
                            ps = psS.tile([P, 512], F32, tag="psS")
                            a = attp.tile([P, 512], BF16, tag="att")
                            if kk < 4 * qc:
                                nc.tensor.matmul(
                                    ps, kTt[h][:, kk * P:(kk + 1) * P], qT[h][:, ts],
                                    start=True, stop=True,
                                )
                                nc.scalar.activation(
                                    a[:], ps[:],
                                    mybir.ActivationFunctionType.Exp, scale=SCALE,
                                )
                            else:
                                off = (kk - 4 * qc) * P
                                nc.tensor.matmul(
                                    ps[:, off:512],
                                    kTt[h][:, kk * P:(kk + 1) * P],
                                    qT[h][:, qc * 512 + off:(qc + 1) * 512],
                                    start=True, stop=True,
                                )
                                nc.scalar.activation(
                                    a[:, off:512], ps[:, off:512],
                                    mybir.ActivationFunctionType.Exp, scale=SCALE,
                                )
                                nc.vector.tensor_tensor(
                                    a[:, off:off + P], a[:, off:off + P], tri,
                                    mybir.AluOpType.mult,
                                )
                            att.append(a)
                        return att

                    # head 0 scores/exp pulled forward: ScalarE works on them
                    # while PE does the v-projections below
                    att0 = attn_scores(0)

                    # ---- Stage A(tc): v projections ----
                    for tt in range(4):
                        pv = psA.tile([P, 512], F32, tag="psA")
                        for kk in range(KT):
                            nc.tensor.matmul(
                                pv, xc[:, kk, tt * P:(tt + 1) * P], wv_sb[:, kk, :],
                                start=(kk == 0), stop=(kk == KT - 1),
                            )
                        nc.vector.tensor_copy(
                            va_all[:, tcu * 4 + tt, :, 0:D_HEAD],
                            pv.rearrange("p (h d) -> p h d", h=NH),
                        )

                    # ---- Stage B(qc=tc): remaining heads + weighted sums ----
                    ytc = []
                    for h in range(NH):
                        att = att0 if h == 0 else attn_scores(h)

                        yt = ytp.tile([P, 512], BF16, tag=f"yT{h}", name=f"yT{h}_{qc}")
                        ytc.append(yt)
                        for j in range(4):
                            qt = qc * 4 + j
                            py = psY.tile([P, VW], F32, tag="psY")
                            for kk in range(qt + 1):
                                nc.tensor.matmul(
                                    py, att[kk][:, j * P:(j + 1) * P],
                                    va_all[:, kk, h, :],
                                    start=(kk == 0), stop=(kk == qt),
                                )
                            r = ynorm.tile([P, 1], F32, tag="r")
                            nc.vector.reciprocal(r, py[:, D_HEAD:VW])
                            y = ynorm.tile([P, P], BF16, tag="y")
                            nc.vector.tensor_scalar_mul(y, py[:, 0:D_HEAD], r)
                            pt = psT.tile([P, P], BF16, tag="psT")
                            nc.tensor.transpose(pt, y, ident)
                            nc.vector.tensor_copy(yt[:, j * P:(j + 1) * P], pt)

                    if prev_ytc is not None:
                        emit_proj(tcu - 1, prev_ytc)
                    prev_ytc = ytc

                emit_proj(TCH - 1, prev_ytc)

    nc.compile()
    return nc


def _get_program() -> bacc.Bacc:
    if "nc" not in _CACHE:
        _CACHE["nc"] = _build_program()
    return _CACHE["nc"]


def _make_in_maps(x, W_attn, W_proj):
    bf = ml_dtypes.bfloat16
    x = np.asarray(x, dtype=np.float32)
    W_attn = np.asarray(W_attn, dtype=np.float32)
    W_proj = np.asarray(W_proj, dtype=np.float32)

    # xT[tc, p, kt, t'] = x[b][tc*512+t', kt*128+p]
    xT_b = []
    for b in range(B):
        xt = x[b].T.reshape(KT, P, TCH, 512).transpose(2, 1, 0, 3)
        xT_b.append(np.ascontiguousarray(xt).astype(bf))

    def _tile_w(w):  # (C, 512) -> (P, KT, 512)
        return np.ascontiguousarray(
            w.reshape(KT, P, NH * P).transpose(1, 0, 2)).astype(bf)

    GW = NH * D_HEAD  # 512 columns per tp group
    in_maps = []
    for core in range(8):
        b, g = divmod(core, 4)
        wp = W_proj[g * GW:(g + 1) * GW, :].reshape(NH, P, C).transpose(1, 0, 2)
        in_maps.append({
            "xT": xT_b[b],
            "wq": _tile_w(W_attn[:, g * GW:(g + 1) * GW]),
            "wk": _tile_w(W_attn[:, C + g * GW: C + (g + 1) * GW]),
            "wv": _tile_w(W_attn[:, 2 * C + g * GW: 2 * C + (g + 1) * GW]),
            "wp": np.ascontiguousarray(wp).astype(bf),
        })
    return in_maps


def kernel(x, W_attn, W_proj, _want_results=False, _trace=False):
    nc = _get_program()
    in_maps = _make_in_maps(x, W_attn, W_proj)
    res = run_bass_kernel_spmd(
        nc, in_maps, core_ids=list(range(8)), trace=_trace,
    )
    parts = [np.asarray(res.results[i]["out"], dtype=np.float32) for i in range(8)]
    out = np.stack([
        parts[0] + parts[1] + parts[2] + parts[3],
        parts[4] + parts[5] + parts[6] + parts[7],
    ]).astype(np.float32)
    if _want_results:
        return out, res
    return out



# revision 4
# speedup vs baseline: 1.8509x; 1.4492x over previous
"""Causal self-attention (B=2, T=2048, C=2048, H=16, Dh=128) on 8 TRN2 NeuronCores.

Sharding: dp=2 over batch x tp=4 over heads (4 heads/core).
  - c_attn column-parallel: each core holds W_attn columns for its 4 heads
    (q, k, v slices), computes qT/kT (head-dim major) and v directly from
    a host-pre-transposed, pre-tiled xT.
  - attention: per-head causal blocks, scoresT in (k, q) orientation; the
    softmax denominator comes free from a ones column appended to v, and
    the exp'd attT blocks feed matmul2 as stationary weights (no
    transposes in the attention inner loop).
  - c_proj row-parallel: each core computes its partial y_heads @ W_proj
    rows; the 4 partials per batch are summed on the host (unshard).

The three stages are software-pipelined per 512-row T-chunk, ordered
q,k-projections -> head-0 scores+exp -> v-projections -> remaining
heads -> deferred c_proj of the previous chunk, so ScalarE exp work
hides under TensorE matmul work of the neighboring sub-stages.

All matmuls run in bf16 (inputs pre-cast on host), accumulation fp32.
Host pre-tiles every input so all DMAs are fully contiguous.
"""

import numpy as np
import ml_dtypes

import concourse.bass as bass
import concourse.tile as tile
from concourse import bacc, mybir
from concourse.bass_utils import run_bass_kernel_spmd
from concourse.masks import make_identity, make_upper_triangular

BF16 = mybir.dt.bfloat16
F32 = mybir.dt.float32

B, T, C = 2, 2048, 2048
N_HEAD, D_HEAD = 16, 128
P = 128
KT = C // P          # 16 contraction tiles for qkv projection
NH = 4               # heads per core (tp=4)
TCH = 4              # T chunks of 512 (pipeline granularity)
VW = 129             # v width with appended ones column
WG = 4               # kt-groups per input DMA (finer grain to start PE early)
SCALE = float(1.0 / np.sqrt(D_HEAD))

_CACHE: dict = {}


def _build_program(repeat: int | None = None) -> bacc.Bacc:
    """Build the SPMD program. With `repeat`, the whole body runs inside a
    hardware For_i loop (used only for timing measurements)."""
    import contextlib
    nc = bacc.Bacc("TRN2", target_bir_lowering=False, debug=False)

    # host-pre-tiled layouts (all DMAs contiguous):
    #   xT:  (TCH, P, KT, 512)  xT[tc, p, kt, t'] = x[b][tc*512+t', kt*128+p]
    #   wq/wk/wv: (P, KT, 512)  w[p, kt, m] = W[kt*128+p, m]
    #   wp:  (P, NH, C)         wp[p, h, n] = W_proj[g*512 + h*128+p, n]
    xT_h = nc.dram_tensor("xT", (TCH, P, KT, 512), BF16, kind="ExternalInput")
    wq_h = nc.dram_tensor("wq", (P, KT, NH * P), BF16, kind="ExternalInput")
    wk_h = nc.dram_tensor("wk", (P, KT, NH * P), BF16, kind="ExternalInput")
    wv_h = nc.dram_tensor("wv", (P, KT, NH * P), BF16, kind="ExternalInput")
    wp_h = nc.dram_tensor("wp", (P, NH, C), BF16, kind="ExternalInput")
    out_h = nc.dram_tensor("out", (T, C), F32, kind="ExternalOutput")

    xT_d = xT_h.ap()
    wq_d, wk_d, wv_d, wp_d = wq_h.ap(), wk_h.ap(), wv_h.ap(), wp_h.ap()
    out_d = out_h.ap().rearrange("(mt p) n -> p mt n", p=P)    # (128, 16, 2048)

    with tile.TileContext(nc) as tc_:
        with (
            tc_.tile_pool(name="consts", bufs=1) as consts,
            tc_.tile_pool(name="persist", bufs=1) as persist,
            tc_.tile_pool(name="wpool", bufs=1) as wpool,
            tc_.tile_pool(name="xpool", bufs=2) as xpool,
            tc_.tile_pool(name="attp", bufs=21) as attp,
            tc_.tile_pool(name="ytp", bufs=2) as ytp,
            tc_.tile_pool(name="ynorm", bufs=3) as ynorm,
            tc_.tile_pool(name="osb", bufs=2) as osb,
            tc_.tile_pool(name="psA", bufs=2, space="PSUM") as psA,
            tc_.tile_pool(name="psS", bufs=2, space="PSUM") as psS,
            tc_.tile_pool(name="psY", bufs=1, space="PSUM") as psY,
            tc_.tile_pool(name="psT", bufs=1, space="PSUM") as psT,
            tc_.tile_pool(name="psO", bufs=2, space="PSUM") as psO,
        ):
            ident = consts.tile([P, P], BF16)
            tri = consts.tile([P, P], BF16)
            make_identity(nc, ident)
            # tri[i, j] = 1 where i <= j (keep k <= q), else 0
            make_upper_triangular(nc, tri, val=1.0, diag=True)

            qT = [persist.tile([P, T], BF16, tag=f"qT{h}", name=f"qT{h}")
                  for h in range(NH)]
            kTt = [persist.tile([P, T], BF16, tag=f"kT{h}", name=f"kT{h}")
                   for h in range(NH)]
            # v_aug for all heads: va_all[:, kk, h, 0:128]=v, [...,128]=1.0
            va_all = persist.tile([P, KT, NH, VW], BF16, tag="va")
            wp_sb = persist.tile([P, NH, C], BF16, tag="wp")

            wq_sb = wpool.tile([P, KT, NH * P], BF16, tag="wq")
            wk_sb = wpool.tile([P, KT, NH * P], BF16, tag="wk")
            wv_sb = wpool.tile([P, KT, NH * P], BF16, tag="wv")

            # first x chunk + q/k weights first (kt-group granularity) so the
            # first matmuls start as early as possible
            xc0 = xpool.tile([P, KT, 512], BF16, tag="xc", name="xc0")
            KG = KT // WG
            for g in range(WG):
                gs = slice(g * KG, (g + 1) * KG)
                nc.sync.dma_start(xc0[:, gs], xT_d[0, :, gs])
                nc.sync.dma_start(wq_sb[:, gs], wq_d[:, gs])
            for g in range(WG):
                gs = slice(g * KG, (g + 1) * KG)
                nc.sync.dma_start(wk_sb[:, gs], wk_d[:, gs])
            for g in range(WG):
                gs = slice(g * KG, (g + 1) * KG)
                nc.sync.dma_start(wv_sb[:, gs], wv_d[:, gs])
            nc.sync.dma_start(wp_sb[:], wp_d[:])
            # ones column for free softmax denominators
            nc.vector.memset(va_all[:, :, :, D_HEAD:VW], 1.0)

            loop_ctx = tc_.For_i(0, repeat, 1) if repeat else contextlib.nullcontext()

            def emit_proj(tcu, ytc):
                # ---- Stage C(tc): c_proj partial for rows of chunk tc ----
                # emitted after B(tc+1) so it backfills PE during exp stalls
                for j in range(4):
                    mt = tcu * 4 + j
                    o = osb.tile([P, C], F32, tag="o", name=f"o_{mt}")
                    for nck in range(4):
                        ns = slice(nck * 512, (nck + 1) * 512)
                        po = psO.tile([P, 512], F32, tag="psO", name=f"psO_{mt}_{nck}")
                        for h in range(NH):
                            nc.tensor.matmul(
                                po, ytc[h][:, j * P:(j + 1) * P], wp_sb[:, h, ns],
                                start=(h == 0), stop=(h == NH - 1),
                            )
                        nc.vector.tensor_copy(o[:, ns], po)
                    nc.sync.dma_start(out_d[:, mt, :], o[:])

            with loop_ctx:
                prev_ytc = None
                xcs = xc0
                for tcu in range(TCH):
                    ts = slice(tcu * 512, (tcu + 1) * 512)
                    xc = xcs
                    if tcu + 1 < TCH:  # prefetch next chunk
                        xcs = xpool.tile([P, KT, 512], BF16, tag="xc",
                                         name=f"xc{tcu + 1}")
                        nc.sync.dma_start(xcs[:], xT_d[tcu + 1])

                    # ---- Stage A(tc): q,k projections first ----
                    for h in range(NH):
                        hs = slice(h * P, (h + 1) * P)
                        pq = psA.tile([P, 512], F32, tag="psA")
                        for kk in range(KT):
                            nc.tensor.matmul(
                                pq, wq_sb[:, kk, hs], xc[:, kk, :],
                                start=(kk == 0), stop=(kk == KT - 1),
                            )
                        nc.vector.tensor_copy(qT[h][:, ts], pq)
                    for h in range(NH):
                        hs = slice(h * P, (h + 1) * P)
                        pk = psA.tile([P, 512], F32, tag="psA")
                        for kk in range(KT):
                            nc.tensor.matmul(
                                pk, wk_sb[:, kk, hs], xc[:, kk, :],
                                start=(kk == 0), stop=(kk == KT - 1),
                            )
                        nc.vector.tensor_copy(kTt[h][:, ts], pk)

                    qc = tcu

                    def attn_scores(h):
                        # mm1 + exp (+ causal mask) for one head's q chunk
                        att = []
                        for kk in range(4 * qc + 4):
                            ps = psS.tile([P, 512], F32, tag="psS")
                            a = attp.tile([P, 512], BF16, tag="att")
                            if kk < 4 * qc:
                                nc.tensor.matmul(
                                    ps, kTt[h][:, kk * P:(kk + 1) * P], qT[h][:, ts],
                                    start=True, stop=True,
                                )
                                nc.scalar.activation(
                                    a[:], ps[:],
                                    mybir.ActivationFunctionType.Exp, scale=SCALE,
                                )
                            else:
                                off = (kk - 4 * qc) * P
                                nc.tensor.matmul(
                                    ps[:, off:512],
                                    kTt[h][:, kk * P:(kk + 1) * P],
                                    qT[h][:, qc * 512 + off:(qc + 1) * 512],
                                    start=True, stop=True,
                                )
                                nc.scalar.activation(
                                    a[:, off:512], ps[:, off:512],
                                    mybir.ActivationFunctionType.Exp, scale=SCALE,
                                )
                                nc.vector.tensor_tensor(
                                    a[:, off:off + P], a[:, off:off + P], tri,
                                    mybir.AluOpType.mult,
                                )
                            att.append(a)
                        return att

                    # head 0 scores/exp pulled forward: ScalarE works on them
                    # while PE does the v-projections below
                    att0 = attn_scores(0)

                    def gen_vproj(tt):
                        # one v-projection column tile; yields per matmul so it
                        # can be woven between att@v matmuls as 512-col filler
                        pv = psA.tile([P, 512], F32, tag="psA",
                                      name=f"pv_{tcu}_{tt}")
                        for kk in range(KT):
                            nc.tensor.matmul(
                                pv, xc[:, kk, tt * P:(tt + 1) * P],
                                wv_sb[:, kk, :],
                                start=(kk == 0), stop=(kk == KT - 1),
                            )
                            yield
                        nc.vector.tensor_copy(
                            va_all[:, tcu * 4 + tt, :, 0:D_HEAD],
                            pv.rearrange("p (h d) -> p h d", h=NH),
                        )

                    def gen_cproj(tcv, ytcv):
                        # c_proj of the previous chunk, yielding per matmul
                        for jmt in range(4):
                            mt = tcv * 4 + jmt
                            o = osb.tile([P, C], F32, tag="o", name=f"o_{mt}")
                            for nck in range(4):
                                ns = slice(nck * 512, (nck + 1) * 512)
                                po = psO.tile([P, 512], F32, tag="psO",
                                              name=f"psO_{mt}_{nck}")
                                for h2 in range(NH):
                                    nc.tensor.matmul(
                                        po, ytcv[h2][:, jmt * P:(jmt + 1) * P],
                                        wp_sb[:, h2, ns],
                                        start=(h2 == 0), stop=(h2 == NH - 1),
                                    )
                                    yield
                                nc.vector.tensor_copy(o[:, ns], po)
                            nc.sync.dma_start(out_d[:, mt, :], o[:])

                    # v tiles whose diag-block consumers come too early to be
                    # fed by fillers are emitted up front; the rest (plus the
                    # deferred c_proj) interleave 1:1 with the narrow att@v
                    # matmuls, hiding their ~139ns issue floor under 512-col
                    # streams (measured: a 512+129 pair costs ~305ns vs 368).
                    import itertools
                    if tcu == 0:
                        up_tt = 4
                    elif tcu == 1:
                        up_tt = 3
                    else:
                        up_tt = 2
                    for tt in range(up_tt):
                        for _ in gen_vproj(tt):
                            pass
                    fills = [gen_vproj(tt) for tt in range(up_tt, 4)]
                    if prev_ytc is not None:
                        fills.append(gen_cproj(tcu - 1, prev_ytc))
                    fillers = itertools.chain(*fills)

                    # ---- Stage B(qc=tc): remaining heads + weighted sums ----
                    ytc = []
                    for h in range(NH):
                        att = att0 if h == 0 else attn_scores(h)

                        yt = ytp.tile([P, 512], BF16, tag=f"yT{h}", name=f"yT{h}_{qc}")
                        ytc.append(yt)
                        for j in range(4):
                            qt = qc * 4 + j
                            py = psY.tile([P, VW], F32, tag="psY")
                            for kk in range(qt + 1):
                                nc.tensor.matmul(
                                    py, att[kk][:, j * P:(j + 1) * P],
                                    va_all[:, kk, h, :],
                                    start=(kk == 0), stop=(kk == qt),
                                )
                                next(fillers, None)
                            r = ynorm.tile([P, 1], F32, tag="r")
                            nc.vector.reciprocal(r, py[:, D_HEAD:VW])
                            y = ynorm.tile([P, P], BF16, tag="y")
                            nc.vector.tensor_scalar_mul(y, py[:, 0:D_HEAD], r)
                            pt = psT.tile([P, P], BF16, tag="psT")
                            nc.tensor.transpose(pt, y, ident)
                            nc.vector.tensor_copy(yt[:, j * P:(j + 1) * P], pt)

                    for _ in fillers:
                        pass
                    prev_ytc = ytc

                emit_proj(TCH - 1, prev_ytc)

    nc.compile()
    return nc


def _get_program() -> bacc.Bacc:
    if "nc" not in _CACHE:
        _CACHE["nc"] = _build_program()
    return _CACHE["nc"]


def _make_in_maps(x, W_attn, W_proj):
    bf = ml_dtypes.bfloat16
    x = np.asarray(x, dtype=np.float32)
    W_attn = np.asarray(W_attn, dtype=np.float32)
    W_proj = np.asarray(W_proj, dtype=np.float32)

    # xT[tc, p, kt, t'] = x[b][tc*512+t', kt*128+p]
    xT_b = []
    for b in range(B):
        xt = x[b].T.reshape(KT, P, TCH, 512).transpose(2, 1, 0, 3)
        xT_b.append(np.ascontiguousarray(xt).astype(bf))

    def _tile_w(w):  # (C, 512) -> (P, KT, 512)
        return np.ascontiguousarray(
            w.reshape(KT, P, NH * P).transpose(1, 0, 2)).astype(bf)

    GW = NH * D_HEAD  # 512 columns per tp group
    in_maps = []
    for core in range(8):
        b, g = divmod(core, 4)
        wp = W_proj[g * GW:(g + 1) * GW, :].reshape(NH, P, C).transpose(1, 0, 2)
        in_maps.append({
            "xT": xT_b[b],
            "wq": _tile_w(W_attn[:, g * GW:(g + 1) * GW]),
            "wk": _tile_w(W_attn[:, C + g * GW: C + (g + 1) * GW]),
            "wv": _tile_w(W_attn[:, 2 * C + g * GW: 2 * C + (g + 1) * GW]),
            "wp": np.ascontiguousarray(wp).astype(bf),
        })
    return in_maps


def kernel(x, W_attn, W_proj, _want_results=False, _trace=False):
    nc = _get_program()
    in_maps = _make_in_maps(x, W_attn, W_proj)
    res = run_bass_kernel_spmd(
        nc, in_maps, core_ids=list(range(8)), trace=_trace,
    )
    parts = [np.asarray(res.results[i]["out"], dtype=np.float32) for i in range(8)]
    out = np.stack([
        parts[0] + parts[1] + parts[2] + parts[3],
        parts[4] + parts[5] + parts[6] + parts[7],
    ]).astype(np.float32)
    if _want_results:
        return out, res
    return out

